# revision 29
# baseline (speedup 1.0000x reference)
"""GAT layer kernel for Trainium2, 8 NeuronCores, edge/node-parallel.

Strategy v2 (degree-balanced node bins, bf16 streams, no dst gather):
  - Host: greedy LPT-pack nodes into 160 bins of <=128 nodes with near-equal
    total in-degree -> every (core, block) has ~E/160 edges, t_b uniform,
    padding ~2.5%.  Edges grouped by dst bin; dstl = local index in bin.
    All node data lives in bin-slot space, rotated per core so each core's
    own 20 bins occupy slots 0..2559 (SPMD program, per-core data).
  - Node phase: stream (permuted) h^T in bf16; per 512-node group compute LN
    stats via PE matmuls, PE-transpose the same tile back to h rows, and emit
    T3 rows [h(128) | 1 | eh | pad] in bf16 (512 B rows; descriptor cost is
    flat below 512B so the row rides free).  et stays on-chip as a bf16
    [128, 160] tile whose column b holds the et of block b's 128 dst nodes.
  - Edge phase per block: er = tanh(LN(r)@w) from a bf16 r^T stream; ONE
    dma_gather of T3 rows by src slot (512 B); et per edge via
    partition_broadcast of the slot-ordered dstl row + is_equal -> transposed
    one-hot -> 1-col matmuls against the resident et column (no dst gather);
    softmax without max-subtraction (exp(relu(x)) == max(1,exp(x)));
    bf16 scaled one-hot in ONE tensor_scalar; PSUM-accumulated bf16 matmul
    onehot^T @ [h|1] gives feat and esum together.
  - Final: feat/esum, feat @ fc_w + b, row L2 normalize, DMA out.
"""

import os
import sys

sys.path.insert(0, "/opt/trn_rl_repo")

_PHASES = int(os.environ.get("KPHASES", "4"))

import heapq

import ml_dtypes
import numpy as np

import concourse.bacc as bacc
import concourse.bass as bass
import concourse.mybir as mybir
import concourse.tile as tile
from concourse.bass_interp import get_hw_module

F32 = mybir.dt.float32
BF16 = mybir.dt.bfloat16
I16 = mybir.dt.int16
AF = mybir.ActivationFunctionType
OP = mybir.AluOpType
NPBF = ml_dtypes.bfloat16

N = 20000
E = 640000
D = 128
NCORES = 8
EPS = 1e-6
NPAD = 20480          # slots: 160 bins * 128
NBIN = 160            # global 128-node bins
NB = NBIN // NCORES   # 20 blocks per core
NSLC = NPAD // NCORES  # slots per core (rotation unit)
TROW = 256            # T3 row: [h(128) | 1 | eh | pad] bf16 (512B)


# ----------------------------------------------------------------- host prep
def _host_prep(h, r, src, dst, hn_a, hn_b, tn_a, tn_b, rn_a, rn_b,
               head_w, tail_w, rel_w, fc_w, fc_b):
    h = np.asarray(h, np.float32); r = np.asarray(r, np.float32)
    src = np.asarray(src, np.int32); dst = np.asarray(dst, np.int32)

    u_h = np.asarray(hn_a, np.float32) * np.asarray(head_w, np.float32)
    u_t = np.asarray(tn_a, np.float32) * np.asarray(tail_w, np.float32)
    u_r = np.asarray(rn_a, np.float32) * np.asarray(rel_w, np.float32)
    s_uh = float(u_h.sum()); s_ut = float(u_t.sum()); s_ur = float(u_r.sum())
    c_h = float((np.asarray(hn_b, np.float32) * head_w).sum())
    c_t = float((np.asarray(tn_b, np.float32) * tail_w).sum())
    c_r = float((np.asarray(rn_b, np.float32) * rel_w).sum())

    # ---- LPT bin packing: 160 bins, <=128 nodes, balanced in-degree
    deg = np.bincount(dst, minlength=N).astype(np.int64)
    order = np.argsort(-deg, kind="stable")
    bin_of = np.empty(N, np.int32)
    loc_of = np.empty(N, np.int32)
    bin_cnt = np.zeros(NBIN, np.int32)
    bin_edges = np.zeros(NBIN, np.int64)
    heap = [(0, b) for b in range(NBIN)]
    heapq.heapify(heap)
    for n in order:
        while True:
            e_b, b = heapq.heappop(heap)
            if bin_cnt[b] < 128:
                break
        bin_of[n] = b
        loc_of[n] = bin_cnt[b]
        bin_cnt[b] += 1
        bin_edges[b] = e_b + deg[n]
        if bin_cnt[b] < 128:
            heapq.heappush(heap, (int(bin_edges[b]), b))
    t_b = max(1, int(-(-int(bin_edges.max()) // 128)))
    e_blk = t_b * 128
    s_b = e_blk // 16
    ep = NB * e_blk

    slot_of = bin_of.astype(np.int64) * 128 + loc_of   # node -> global slot
    node_of = np.zeros(NPAD, np.int64)                 # slot -> node (pad: 0)
    node_of[slot_of] = np.arange(N)

    # ---- edges grouped by dst bin
    eb = bin_of[dst]
    perm = np.argsort(eb, kind="stable")
    src_s = src[perm]; dst_s = dst[perm]
    counts = np.bincount(eb, minlength=NBIN)
    cum = np.concatenate([[0], np.cumsum(counts)])

    # ---- replicated tensors
    h_slot = h[node_of]                                # [NPAD, D] slot space
    hT_slot = np.ascontiguousarray(h_slot.T.astype(NPBF))  # [128, NPAD]
    iota = np.broadcast_to(np.arange(128, dtype=np.float32),
                           (128, 128)).astype(NPBF).copy()
    iotac = np.arange(128, dtype=np.float32).reshape(128, 1)
    ident = np.eye(128, dtype=np.float32).astype(NPBF)
    wn = np.zeros((128, 4), np.float32)
    wn[:, 0] = 1.0; wn[:, 1] = u_h; wn[:, 2] = u_t
    wn = wn.astype(NPBF)
    wr = np.zeros((128, 2), np.float32)
    wr[:, 0] = 1.0; wr[:, 1] = u_r
    wr = wr.astype(NPBF)
    fcw = np.ascontiguousarray(np.asarray(fc_w, np.float32)).astype(NPBF)
    fcb = np.broadcast_to(np.asarray(fc_b, np.float32), (128, 128)).copy()
    consts = np.zeros((128, 8), np.float32)
    consts[:, 0] = s_uh; consts[:, 1] = s_ut; consts[:, 2] = s_ur
    consts[:, 3] = c_h; consts[:, 4] = c_t; consts[:, 5] = c_r

    rep = {"iota": iota, "iotac": iotac, "ident": ident, "wn": wn, "wr": wr,
           "fcw": fcw, "fcb": fcb, "consts": consts}

    in_maps = []
    for k in range(NCORES):
        src16 = np.zeros((NB, e_blk), np.int16)
        dstl = np.full((NB, e_blk), 200.0, np.float32)
        rcol = np.zeros((NB, e_blk), np.int64)
        for j in range(NB):
            b = k * NB + j
            e0, e1 = int(cum[b]), int(cum[b + 1])
            cnt = e1 - e0
            # src slot in core-k-rotated space
            src16[j, :cnt] = (slot_of[src_s[e0:e1]] - NSLC * k) % NPAD
            dstl[j, :cnt] = loc_of[dst_s[e0:e1]]
            rcol[j, :cnt] = perm[e0:e1]
        rT = np.ascontiguousarray(r[rcol.reshape(-1)].T).astype(NPBF)
        hT = np.ascontiguousarray(np.roll(hT_slot, -NSLC * k, axis=1))

        def wrap16(a):
            blk = a.reshape(NB, s_b, 16).transpose(0, 2, 1)    # [NB,16,s_b]
            out = np.tile(blk, (1, 8, 1))                      # [NB,128,s_b]
            return np.ascontiguousarray(
                out.transpose(1, 0, 2).reshape(128, NB * s_b))

        def pk(a):
            x = a.reshape(NB, t_b, 128).transpose(2, 0, 1)     # [128, NB, t_b]
            return np.ascontiguousarray(x.reshape(128, NB * t_b))

        in_maps.append(dict(rep, rT=rT, hT=hT, idx_src=wrap16(src16),
                            dstl=pk(dstl),
                            dstl_row=dstl.reshape(1, NB * e_blk).astype(NPBF)))
    meta = dict(t_b=t_b, e_blk=e_blk, s_b=s_b, ep=ep,
                bin_of=bin_of, loc_of=loc_of)
    return in_maps, meta


# ------------------------------------------------------------ device program
def build_program(t_b, loop_k=1, for_hw=True):
    e_blk = t_b * 128
    s_b = e_blk // 16
    ep = NB * e_blk
    nc = bacc.Bacc("TRN2", target_bir_lowering=False, debug=False,
                   enable_asserts=False, num_devices=NCORES if for_hw else 1)

    dt_rT = nc.dram_tensor("rT", [128, ep], BF16, kind="ExternalInput")
    dt_hT = nc.dram_tensor("hT", [128, NPAD], BF16, kind="ExternalInput")
    dt_isrc = nc.dram_tensor("idx_src", [128, NB * s_b], I16, kind="ExternalInput")
    dt_dstl = nc.dram_tensor("dstl", [128, NB * t_b], F32, kind="ExternalInput")
    dt_dstlr = nc.dram_tensor("dstl_row", [1, NB * e_blk], BF16, kind="ExternalInput")
    dt_iota = nc.dram_tensor("iota", [128, 128], BF16, kind="ExternalInput")
    dt_iotac = nc.dram_tensor("iotac", [128, 1], F32, kind="ExternalInput")
    dt_ident = nc.dram_tensor("ident", [128, 128], BF16, kind="ExternalInput")
    dt_wn = nc.dram_tensor("wn", [128, 4], BF16, kind="ExternalInput")
    dt_wr = nc.dram_tensor("wr", [128, 2], BF16, kind="ExternalInput")
    dt_fcw = nc.dram_tensor("fcw", [128, 128], BF16, kind="ExternalInput")
    dt_fcb = nc.dram_tensor("fcb", [128, 128], F32, kind="ExternalInput")
    dt_consts = nc.dram_tensor("consts", [128, 8], F32, kind="ExternalInput")
    dt_out = nc.dram_tensor("out", [NB * 128, 128], F32, kind="ExternalOutput")
    dt_T3 = nc.dram_tensor("T3", [NPAD, TROW], BF16, kind="ExternalOutput")

    NG = NPAD // 512          # node-phase groups
    NPK = NPAD // 128         # node cols (slot space)
    EPK = NB * t_b            # packed edge cols

    with tile.TileContext(nc) as tc:
        with tc.tile_pool(name="const", bufs=1) as cpool:
            iota_sb = cpool.tile([128, 128], BF16)
            nc.sync.dma_start(out=iota_sb[:], in_=dt_iota.ap())
            iotac_sb = cpool.tile([128, 1], F32)
            nc.sync.dma_start(out=iotac_sb[:], in_=dt_iotac.ap())
            ident_sb = cpool.tile([128, 128], BF16)
            nc.sync.dma_start(out=ident_sb[:], in_=dt_ident.ap())
            wn_sb = cpool.tile([128, 4], BF16)
            nc.sync.dma_start(out=wn_sb[:], in_=dt_wn.ap())
            wr_sb = cpool.tile([128, 2], BF16)
            nc.sync.dma_start(out=wr_sb[:], in_=dt_wr.ap())
            fcw_sb = cpool.tile([128, 128], BF16)
            nc.sync.dma_start(out=fcw_sb[:], in_=dt_fcw.ap())
            fcb_sb = cpool.tile([128, 128], F32)
            nc.sync.dma_start(out=fcb_sb[:], in_=dt_fcb.ap())
            cst = cpool.tile([128, 8], F32)
            nc.sync.dma_start(out=cst[:], in_=dt_consts.ap())
            isrc_sb = cpool.tile([128, NB * s_b], I16)
            nc.sync.dma_start(out=isrc_sb[:], in_=dt_isrc.ap())
            dstl_sb = cpool.tile([128, NB * t_b], F32)
            nc.sync.dma_start(out=dstl_sb[:], in_=dt_dstl.ap())
            etp16 = cpool.tile([128, NPK], BF16)   # et by slot, col-major
            ones_row = cpool.tile([1, 128], BF16)
            nc.vector.memset(ones_row[:], 1.0)

            def loop_body():
                # ================== node phase: stats + T3 rows ==============
                # per 512-slot group: PE stats matmuls (lhsT=hT slice), PE
                # transposes of the same tile -> h rows -> T3 [h|1] cols;
                # LN finish batched once (single Sqrt: act-table peace), eh
                # lands in T3 col 129 via one strided column DMA; et stays
                # on-chip (etp16).
                with tc.tile_pool(name="nstat", bufs=1) as spool, \
                     tc.tile_pool(name="nwork", bufs=3) as wpool, \
                     tc.tile_pool(name="npsum", bufs=2, space="PSUM") as pp:
                    spk = spool.tile([128, NPK, 4], F32)
                    hTN = spool.tile([128, NPAD], BF16)
                    GW = 2048
                    CG = GW // 128
                    for g in range(NPAD // GW):
                        hTg = hTN[:, GW * g:GW * (g + 1)]
                        nc.sync.dma_start(out=hTg, in_=dt_hT.ap()[:, GW * g:GW * (g + 1)])
                        psS = pp.tile([128, CG, 4], F32, tag="psS")
                        for c in range(CG):
                            nc.tensor.matmul(psS[:, c, 0:3],
                                             hTg[:, 128 * c:128 * (c + 1)],
                                             wn_sb[:, 0:3], start=True, stop=True)
                        sq = wpool.tile([128, GW], BF16, tag="sq")
                        nc.scalar.activation(out=sq[:], in_=hTg, func=AF.Square)
                        for c in range(CG):
                            nc.tensor.matmul(psS[:, c, 3:4],
                                             sq[:, 128 * c:128 * (c + 1)],
                                             wn_sb[:, 0:1], start=True, stop=True)
                        nc.vector.tensor_copy(out=spk[:, CG * g:CG * (g + 1), :],
                                              in_=psS[:])
                    # batched LN finish -> eh (T3 col 129), et (on-chip bf16)
                    s1p = spk[:, :, 0]; suh = spk[:, :, 1]
                    sut = spk[:, :, 2]; s2p = spk[:, :, 3]
                    mu = spool.tile([128, NPK], F32)
                    nc.vector.tensor_scalar_mul(out=mu[:], in0=s1p, scalar1=1.0 / 128.0)
                    t0 = spool.tile([128, NPK], F32)
                    nc.vector.tensor_mul(out=t0[:], in0=mu[:], in1=mu[:])
                    nc.vector.tensor_scalar_mul(out=t0[:], in0=t0[:], scalar1=-128.0)
                    nc.vector.tensor_add(out=t0[:], in0=t0[:], in1=s2p)
                    rstd = spool.tile([128, NPK], F32)
                    nc.scalar.activation(out=rstd[:], in_=t0[:], func=AF.Sqrt, scale=1.0 / 127.0)
                    nc.vector.tensor_scalar_add(out=rstd[:], in0=rstd[:], scalar1=EPS)
                    nc.vector.reciprocal(out=rstd[:], in_=rstd[:])
                    ehc = spool.tile([128, NPK], BF16)
                    for su, sidx, cidx, dst16 in ((suh, 0, 3, ehc), (sut, 1, 4, etp16)):
                        m1 = spool.tile([128, NPK], F32, tag="m1")
                        nc.vector.tensor_scalar_mul(out=m1[:], in0=mu[:], scalar1=cst[:, sidx:sidx + 1])
                        nc.vector.tensor_sub(out=m1[:], in0=su, in1=m1[:])
                        nc.vector.tensor_mul(out=m1[:], in0=m1[:], in1=rstd[:])
                        nc.vector.tensor_scalar_add(out=m1[:], in0=m1[:], scalar1=cst[:, cidx:cidx + 1])
                        nc.scalar.activation(out=dst16[:], in_=m1[:], func=AF.Tanh)
                    for g in range(NPAD // GW):
                        psT = pp.tile([128, CG, 128], BF16, tag="psT")
                        for c in range(CG):
                            nc.tensor.transpose(psT[:, c, :],
                                                hTN[:, GW * g + 128 * c:GW * g + 128 * (c + 1)],
                                                ident_sb[:])
                        hrow = wpool.tile([128, CG, 129], BF16, tag="hrow")
                        nc.vector.tensor_copy(out=hrow[:, :, 0:128], in_=psT[:])
                        nc.vector.tensor_copy(out=hrow[:, :, 128],
                                              in_=ehc[:, CG * g:CG * (g + 1)])
                        nc.sync.dma_start(
                            out=dt_T3.ap()[GW * g:GW * (g + 1), 0:129]
                                .rearrange("(c p) w -> p c w", p=128),
                            in_=hrow[:, :, :])
                    if _PHASES == 1:
                        return

                # ================= edge phase 1: er stats ====================
                with tc.tile_pool(name="estat", bufs=1) as espool:
                    epk3 = espool.tile([128, EPK, 3], F32)
                    erp = espool.tile([128, EPK], F32)
                    with tc.tile_pool(name="ework", bufs=2) as ewpool, \
                         tc.tile_pool(name="epsum", bufs=2, space="PSUM") as epp:
                        for b in range(NB):
                            rTb = ewpool.tile([128, e_blk], BF16, tag="rTb")
                            nc.sync.dma_start(out=rTb[:], in_=dt_rT.ap()[:, b * e_blk:(b + 1) * e_blk])
                            psE = epp.tile([128, 3 * t_b], F32, tag="psE")
                            for t in range(t_b):
                                nc.tensor.matmul(psE[:, 3 * t:3 * t + 2],
                                                 rTb[:, 128 * t:128 * (t + 1)],
                                                 wr_sb[:], start=True, stop=True)
                            if b % 2 == 0:
                                nc.scalar.activation(out=rTb[:], in_=rTb[:], func=AF.Square)
                            else:
                                nc.vector.tensor_mul(out=rTb[:], in0=rTb[:], in1=rTb[:])
                            for t in range(t_b):
                                nc.tensor.matmul(psE[:, 3 * t + 2:3 * t + 3],
                                                 rTb[:, 128 * t:128 * (t + 1)],
                                                 wr_sb[:, 0:1], start=True, stop=True)
                            nc.scalar.activation(out=epk3[:, b * t_b:(b + 1) * t_b, :],
                                                 in_=psE[:], func=AF.Copy)
                    # batched er finish (strided stat views)
                    s1e = epk3[:, :, 0]; sue = epk3[:, :, 1]; s2e = epk3[:, :, 2]
                    mu = espool.tile([128, EPK], F32)
                    nc.vector.tensor_scalar_mul(out=mu[:], in0=s1e, scalar1=1.0 / 128.0)
                    t0 = espool.tile([128, EPK], F32)
                    nc.vector.tensor_mul(out=t0[:], in0=mu[:], in1=mu[:])
                    nc.vector.tensor_scalar_mul(out=t0[:], in0=t0[:], scalar1=-128.0)
                    nc.vector.tensor_add(out=t0[:], in0=t0[:], in1=s2e)
                    rstd = espool.tile([128, EPK], F32)
                    nc.scalar.activation(out=rstd[:], in_=t0[:], func=AF.Sqrt, scale=1.0 / 127.0)
                    nc.vector.tensor_scalar_add(out=rstd[:], in0=rstd[:], scalar1=EPS)
                    nc.vector.reciprocal(out=rstd[:], in_=rstd[:])
                    m1 = espool.tile([128, EPK], F32)
                    nc.vector.tensor_scalar_mul(out=m1[:], in0=mu[:], scalar1=cst[:, 2:3])
                    nc.vector.tensor_sub(out=m1[:], in0=sue, in1=m1[:])
                    nc.vector.tensor_mul(out=m1[:], in0=m1[:], in1=rstd[:])
                    nc.vector.tensor_scalar_add(out=m1[:], in0=m1[:], scalar1=cst[:, 5:6])
                    nc.scalar.activation(out=erp[:], in_=m1[:], func=AF.Tanh)
                    if _PHASES == 2:
                        nc.sync.dma_start(out=dt_out.ap()[0:128, 0:min(EPK, 128)],
                                          in_=erp[:, 0:min(EPK, 128)])
                        return

                    # ===== edge phase 2: per-block A (et expand) + B + final =====
                    with tc.tile_pool(name="gwork", bufs=3) as gpool, \
                         tc.tile_pool(name="feat", bufs=1) as fpool, \
                         tc.tile_pool(name="fin", bufs=2) as npool, \
                         tc.tile_pool(name="bpsum", bufs=2, space="PSUM") as bpp, \
                         tc.tile_pool(name="dpsum", bufs=1, space="PSUM") as dpp, \
                         tc.tile_pool(name="fpsum", bufs=2, space="PSUM") as fpp, \
                         tc.tile_pool(name="finps", bufs=1, space="PSUM") as npp:
                        featst = fpool.tile([128, NB * 129], F32)
                        obst = fpool.tile([128, NB, 128], F32)
                        nr2 = fpool.tile([128, NB], F32)
                        for b in range(NB):
                            # -- A(b): et per edge (er-independent)
                            dstlrb = gpool.tile([1, e_blk], BF16, tag="dstlrb")
                            nc.sync.dma_start(out=dstlrb[:],
                                              in_=dt_dstlr.ap()[:, b * e_blk:(b + 1) * e_blk])
                            ohT = gpool.tile([128, e_blk], BF16, tag="ohT")
                            for q in range(e_blk // 512):
                                psB = bpp.tile([128, 512], F32, tag="psB")
                                nc.tensor.matmul(psB[:], ones_row[:],
                                                 dstlrb[:, 512 * q:512 * (q + 1)],
                                                 start=True, stop=True)
                                nc.vector.tensor_scalar(out=ohT[:, 512 * q:512 * (q + 1)],
                                                        in0=psB[:],
                                                        scalar1=iotac_sb[:, 0:1],
                                                        scalar2=None, op0=OP.is_equal)
                            psD = dpp.tile([128, t_b], F32, tag="psD")
                            for t in range(t_b):
                                nc.tensor.matmul(psD[:, t:t + 1],
                                                 ohT[:, 128 * t:128 * (t + 1)],
                                                 etp16[:, b:b + 1], start=True, stop=True)
                            # -- B(b): gather + exe + scatter matmuls
                            tg = gpool.tile([128, t_b, TROW], BF16, tag="tg")
                            nc.gpsimd.dma_gather(
                                out_ap=tg[:, :, :], in_ap=dt_T3.ap(),
                                idxs_ap=isrc_sb[:, b * s_b:(b + 1) * s_b],
                                num_idxs=e_blk, num_idxs_reg=e_blk, elem_size=TROW,
                                single_packet=False)
                            # exe = exp(relu(eh_src + et_dst + er)); padded slots
                            # self-mask via out-of-range dstl (one-hot row = 0)
                            ex = gpool.tile([128, t_b], F32, tag="ex")
                            nc.vector.tensor_tensor(out=ex[:], in0=tg[:, :, 128],
                                                    in1=erp[:, b * t_b:(b + 1) * t_b],
                                                    op=OP.add)
                            nc.vector.tensor_add(out=ex[:], in0=ex[:], in1=psD[:, :])
                            nc.scalar.activation(out=ex[:], in_=ex[:], func=AF.Relu)
                            nc.scalar.activation(out=ex[:], in_=ex[:], func=AF.Exp)
                            if _PHASES in (23, 24, 25, 26) and b < 4:
                                dbg = gpool.tile([128, t_b], F32, tag="dbg")
                                if _PHASES == 23:
                                    nc.vector.tensor_copy(out=dbg[:], in_=tg[:, :, 128])
                                elif _PHASES == 24:
                                    nc.vector.tensor_copy(out=dbg[:], in_=psD[:, :])
                                elif _PHASES == 26:
                                    nc.vector.tensor_copy(out=dbg[:], in_=tg[:, 0, 0:t_b])
                                else:
                                    nc.vector.tensor_copy(out=dbg[:], in_=ex[:])
                                nc.sync.dma_start(
                                    out=dt_out.ap()[0:128, b * t_b:(b + 1) * t_b],
                                    in_=dbg[:])
                            psF = fpp.tile([128, 128], F32, tag="psF")
                            psS1 = dpp.tile([128, 1], F32, tag="psS1")
                            for t in range(t_b):
                                oh = gpool.tile([128, 128], BF16, tag="oh")
                                nc.vector.tensor_scalar(
                                    out=oh[:], in0=iota_sb[:],
                                    scalar1=dstl_sb[:, b * t_b + t:b * t_b + t + 1],
                                    scalar2=ex[:, t:t + 1],
                                    op0=OP.is_equal, op1=OP.mult)
                                nc.tensor.matmul(psF[:], oh[:], tg[:, t, 0:128],
                                                 start=(t == 0), stop=(t == t_b - 1))
                                nc.tensor.matmul(psS1[:], oh[:], wn_sb[:, 0:1],
                                                 start=(t == 0), stop=(t == t_b - 1))
                            nc.scalar.activation(out=featst[:, b * 129:b * 129 + 128],
                                                 in_=psF[:], func=AF.Copy)
                            nc.scalar.activation(out=featst[:, b * 129 + 128:(b + 1) * 129],
                                                 in_=psS1[:], func=AF.Copy)
                            # -- final(b): feat/esum, fc, norm^2
                            rs = npool.tile([128, 1], F32, tag="rs")
                            nc.vector.tensor_scalar(out=rs[:], in0=featst[:, b * 129 + 128:b * 129 + 129],
                                                    scalar1=1e-30, scalar2=None, op0=OP.max)
                            nc.vector.reciprocal(out=rs[:], in_=rs[:])
                            fs = npool.tile([128, 128], BF16, tag="fs")
                            nc.vector.tensor_scalar_mul(
                                out=fs[:], in0=featst[:, b * 129:b * 129 + 128], scalar1=rs[:])
                            psT = npp.tile([128, 128], BF16, tag="psT")
                            nc.tensor.transpose(psT[:], fs[:], ident_sb[:])
                            fT = npool.tile([128, 128], BF16, tag="fT")
                            nc.scalar.activation(out=fT[:], in_=psT[:], func=AF.Copy)
                            psO = npp.tile([128, 128], F32, tag="psO")
                            nc.tensor.matmul(psO[:], fT[:], fcw_sb[:], start=True, stop=True)
                            nc.vector.tensor_add(out=obst[:, b, :], in0=psO[:], in1=fcb_sb[:])
                            scr = npool.tile([128, 128], F32, tag="scr")
                            nc.vector.tensor_mul(out=scr[:], in0=obst[:, b, :], in1=obst[:, b, :])
                            nc.vector.reduce_sum(out=nr2[:, b:b + 1], in_=scr[:],
                                                 axis=mybir.AxisListType.X)
                        if _PHASES == 3:
                            nc.sync.dma_start(out=dt_out.ap()[0:128, 0:128],
                                              in_=featst[:, 0:128])
                            return
                        # ============== batched L2 normalize + out ==============
                        nrm = fpool.tile([128, NB], F32)
                        nc.scalar.activation(out=nrm[:], in_=nr2[:], func=AF.Sqrt)
                        nc.vector.tensor_scalar(out=nrm[:], in0=nrm[:],
                                                scalar1=1e-12, scalar2=None, op0=OP.max)
                        nc.vector.reciprocal(out=nrm[:], in_=nrm[:])
                        for b in range(NB):
                            ob = npool.tile([128, 128], F32, tag="ob")
                            nc.vector.tensor_scalar_mul(out=ob[:], in0=obst[:, b, :],
                                                        scalar1=nrm[:, b:b + 1])
                            nc.sync.dma_start(out=dt_out.ap()[b * 128:(b + 1) * 128, :], in_=ob[:])

            if loop_k == 1:
                loop_body()
            else:
                with tc.For_i(0, loop_k, 1):
                    loop_body()

    nc.compile()
    if for_hw:
        nc.m = get_hw_module(nc.m)
    return nc


# ------------------------------------------------------------------- runner
class Runner:
    def __init__(self, nc, n_cores=NCORES):
        import jax
        from concourse.bass2jax import (_bass_exec_p, partition_id_tensor,
                                        install_neuronx_cc_hook)
        from jax.sharding import Mesh, PartitionSpec, NamedSharding
        from jax.experimental.shard_map import shard_map
        install_neuronx_cc_hook()
        self.jax = jax
        self.n_cores = n_cores
        pname = nc.partition_id_tensor.name if nc.partition_id_tensor else None
        in_names, out_names, out_avals = [], [], []
        for alloc in nc.m.functions[0].allocations:
            if not isinstance(alloc, mybir.MemoryLocationSet):
                continue
            name = alloc.memorylocations[0].name
            if alloc.kind == "ExternalInput":
                if name != pname:
                    in_names.append(name)
            elif alloc.kind == "ExternalOutput":
                out_names.append(name)
                out_avals.append(jax.core.ShapedArray(
                    tuple(alloc.tensor_shape), mybir.dt.np(alloc.dtype)))
        self.in_names, self.out_names, self.out_avals = in_names, out_names, out_avals
        n_params = len(in_names)
        all_in = list(in_names) + list(out_names)
        if pname is not None:
            all_in.append(pname)

        def _body(*args):
            operands = list(args)
            if pname is not None:
                operands.append(partition_id_tensor())
            return tuple(_bass_exec_p.bind(
                *operands, out_avals=tuple(out_avals), in_names=tuple(all_in),
                out_names=tuple(out_names), lowering_input_output_aliases=(),
                sim_require_finite=True, sim_require_nnan=True, nc=nc))

        devices = jax.devices()[:n_cores]
        self.mesh = Mesh(np.asarray(devices), ("core",))
        self.sharding = NamedSharding(self.mesh, PartitionSpec("core"))
        donate = tuple(range(n_params, n_params + len(out_names)))
        self.fn = jax.jit(shard_map(
            _body, mesh=self.mesh,
            in_specs=(PartitionSpec("core"),) * (n_params + len(out_names)),
            out_specs=(PartitionSpec("core"),) * len(out_names),
            check_rep=False), donate_argnums=donate, keep_unused=True)

    def put_inputs(self, in_maps):
        return [self.jax.device_put(
            np.concatenate([np.asarray(in_maps[c][nm]) for c in range(self.n_cores)], axis=0),
            self.sharding) for nm in self.in_names]

    def put_zeros(self):
        return [self.jax.device_put(
            np.zeros((self.n_cores * a.shape[0], *a.shape[1:]), a.dtype), self.sharding)
            for a in self.out_avals]

    def run(self, dev_in, dev_zeros):
        outs = self.fn(*dev_in, *dev_zeros)
        self.jax.block_until_ready(outs)
        return outs

    def unpack(self, outs):
        return [{nm: np.asarray(outs[i]).reshape(self.n_cores, *self.out_avals[i].shape)[c]
                 for i, nm in enumerate(self.out_names)} for c in range(self.n_cores)]


_CACHE = {}


def _get_runner(t_b, loop_k=1):
    key = (t_b, loop_k)
    if key not in _CACHE:
        nc = build_program(t_b, loop_k)
        _CACHE[key] = Runner(nc)
    return _CACHE[key]


def kernel(**inputs):
    in_maps, meta = _host_prep(**inputs)
    r = _get_runner(meta["t_b"], 1)
    dev = r.put_inputs(in_maps)
    res = r.unpack(r.run(dev, r.put_zeros()))
    bin_of, loc_of = meta["bin_of"], meta["loc_of"]
    slots = bin_of.astype(np.int64) * 128 + loc_of      # global slot per node
    allout = np.concatenate([res[k]["out"] for k in range(NCORES)], axis=0)
    return np.ascontiguousarray(allout[slots])


# revision 30
# speedup vs baseline: 2.0253x; 2.0253x over previous
"""GAT layer kernel for Trainium2, 8 NeuronCores, edge/node-parallel.

Strategy v2 (degree-balanced node bins, bf16 streams, no dst gather):
  - Host: greedy LPT-pack nodes into 160 bins of <=128 nodes with near-equal
    total in-degree -> every (core, block) has ~E/160 edges, t_b uniform,
    padding ~2.5%.  Edges grouped by dst bin; dstl = local index in bin.
    All node data lives in bin-slot space, rotated per core so each core's
    own 20 bins occupy slots 0..2559 (SPMD program, per-core data).
  - Node phase: stream (permuted) h^T in bf16; per 512-node group compute LN
    stats via PE matmuls, PE-transpose the same tile back to h rows, and emit
    T3 rows [h(128) | 1 | eh | pad] in bf16 (512 B rows; descriptor cost is
    flat below 512B so the row rides free).  et stays on-chip as a bf16
    [128, 160] tile whose column b holds the et of block b's 128 dst nodes.
  - Edge phase per block: er = tanh(LN(r)@w) from a bf16 r^T stream; ONE
    dma_gather of T3 rows by src slot (512 B); et per edge via
    partition_broadcast of the slot-ordered dstl row + is_equal -> transposed
    one-hot -> 1-col matmuls against the resident et column (no dst gather);
    softmax without max-subtraction (exp(relu(x)) == max(1,exp(x)));
    bf16 scaled one-hot in ONE tensor_scalar; PSUM-accumulated bf16 matmul
    onehot^T @ [h|1] gives feat and esum together.
  - Final: feat/esum, feat @ fc_w + b, row L2 normalize, DMA out.
"""

import os
import sys

sys.path.insert(0, "/opt/trn_rl_repo")

_PHASES = int(os.environ.get("KPHASES", "4"))

import heapq

import ml_dtypes
import numpy as np

import concourse.bacc as bacc
import concourse.bass as bass
import concourse.mybir as mybir
import concourse.tile as tile
from concourse.bass_interp import get_hw_module

F32 = mybir.dt.float32
BF16 = mybir.dt.bfloat16
I16 = mybir.dt.int16
AF = mybir.ActivationFunctionType
OP = mybir.AluOpType
NPBF = ml_dtypes.bfloat16

N = 20000
E = 640000
D = 128
NCORES = 8
EPS = 1e-6
NPAD = 20480          # slots: 160 bins * 128
NBIN = 160            # global 128-node bins
NB = NBIN // NCORES   # 20 blocks per core
NSLC = NPAD // NCORES  # slots per core (rotation unit)
TROW = 256            # T3 row: [h(128) | 1 | eh | pad] bf16 (512B)


# ----------------------------------------------------------------- host prep
def _host_prep(h, r, src, dst, hn_a, hn_b, tn_a, tn_b, rn_a, rn_b,
               head_w, tail_w, rel_w, fc_w, fc_b):
    h = np.asarray(h, np.float32); r = np.asarray(r, np.float32)
    src = np.asarray(src, np.int32); dst = np.asarray(dst, np.int32)

    u_h = np.asarray(hn_a, np.float32) * np.asarray(head_w, np.float32)
    u_t = np.asarray(tn_a, np.float32) * np.asarray(tail_w, np.float32)
    u_r = np.asarray(rn_a, np.float32) * np.asarray(rel_w, np.float32)
    s_uh = float(u_h.sum()); s_ut = float(u_t.sum()); s_ur = float(u_r.sum())
    c_h = float((np.asarray(hn_b, np.float32) * head_w).sum())
    c_t = float((np.asarray(tn_b, np.float32) * tail_w).sum())
    c_r = float((np.asarray(rn_b, np.float32) * rel_w).sum())

    # ---- LPT bin packing: 160 bins, <=128 nodes, balanced in-degree
    deg = np.bincount(dst, minlength=N).astype(np.int64)
    order = np.argsort(-deg, kind="stable")
    bin_of = np.empty(N, np.int32)
    loc_of = np.empty(N, np.int32)
    bin_cnt = np.zeros(NBIN, np.int32)
    bin_edges = np.zeros(NBIN, np.int64)
    heap = [(0, b) for b in range(NBIN)]
    heapq.heapify(heap)
    for n in order:
        while True:
            e_b, b = heapq.heappop(heap)
            if bin_cnt[b] < 128:
                break
        bin_of[n] = b
        loc_of[n] = bin_cnt[b]
        bin_cnt[b] += 1
        bin_edges[b] = e_b + deg[n]
        if bin_cnt[b] < 128:
            heapq.heappush(heap, (int(bin_edges[b]), b))
    t_b = max(1, int(-(-int(bin_edges.max()) // 128)))
    e_blk = t_b * 128
    s_b = e_blk // 16
    ep = NB * e_blk

    slot_of = bin_of.astype(np.int64) * 128 + loc_of   # node -> global slot
    node_of = np.zeros(NPAD, np.int64)                 # slot -> node (pad: 0)
    node_of[slot_of] = np.arange(N)

    # ---- edges grouped by dst bin
    eb = bin_of[dst]
    perm = np.argsort(eb, kind="stable")
    src_s = src[perm]; dst_s = dst[perm]
    counts = np.bincount(eb, minlength=NBIN)
    cum = np.concatenate([[0], np.cumsum(counts)])

    # ---- replicated tensors
    h_slot = h[node_of]                                # [NPAD, D] slot space
    hT_slot = np.ascontiguousarray(h_slot.T.astype(NPBF))  # [128, NPAD]
    iota = np.broadcast_to(np.arange(128, dtype=np.float32),
                           (128, 128)).astype(NPBF).copy()
    iotac = np.arange(128, dtype=np.float32).reshape(128, 1)
    ident = np.eye(128, dtype=np.float32).astype(NPBF)
    wn = np.zeros((128, 4), np.float32)
    wn[:, 0] = 1.0; wn[:, 1] = u_h; wn[:, 2] = u_t
    wn = wn.astype(NPBF)
    wr = np.zeros((128, 2), np.float32)
    wr[:, 0] = 1.0; wr[:, 1] = u_r
    wr = wr.astype(NPBF)
    fcw = np.ascontiguousarray(np.asarray(fc_w, np.float32)).astype(NPBF)
    fcb = np.broadcast_to(np.asarray(fc_b, np.float32), (128, 128)).copy()
    consts = np.zeros((128, 8), np.float32)
    consts[:, 0] = s_uh; consts[:, 1] = s_ut; consts[:, 2] = s_ur
    consts[:, 3] = c_h; consts[:, 4] = c_t; consts[:, 5] = c_r

    rep = {"iota": iota, "iotac": iotac, "ident": ident, "wn": wn, "wr": wr,
           "fcw": fcw, "fcb": fcb, "consts": consts}

    in_maps = []
    for k in range(NCORES):
        src16 = np.zeros((NB, e_blk), np.int16)
        dstl = np.full((NB, e_blk), 200.0, np.float32)
        rcol = np.zeros((NB, e_blk), np.int64)
        for j in range(NB):
            b = k * NB + j
            e0, e1 = int(cum[b]), int(cum[b + 1])
            cnt = e1 - e0
            # src slot in core-k-rotated space
            src16[j, :cnt] = (slot_of[src_s[e0:e1]] - NSLC * k) % NPAD
            dstl[j, :cnt] = loc_of[dst_s[e0:e1]]
            rcol[j, :cnt] = perm[e0:e1]
        rT = np.ascontiguousarray(r[rcol.reshape(-1)].T).astype(NPBF)
        hT = np.ascontiguousarray(np.roll(hT_slot, -NSLC * k, axis=1))

        def wrap16(a):
            blk = a.reshape(NB, s_b, 16).transpose(0, 2, 1)    # [NB,16,s_b]
            out = np.tile(blk, (1, 8, 1))                      # [NB,128,s_b]
            return np.ascontiguousarray(
                out.transpose(1, 0, 2).reshape(128, NB * s_b))

        def pk(a):
            x = a.reshape(NB, t_b, 128).transpose(2, 0, 1)     # [128, NB, t_b]
            return np.ascontiguousarray(x.reshape(128, NB * t_b))

        in_maps.append(dict(rep, rT=rT, hT=hT, idx_src=wrap16(src16),
                            dstl=pk(dstl),
                            dstl_row=dstl.reshape(1, NB * e_blk).astype(NPBF)))
    meta = dict(t_b=t_b, e_blk=e_blk, s_b=s_b, ep=ep,
                bin_of=bin_of, loc_of=loc_of)
    return in_maps, meta


# ------------------------------------------------------------ device program
def build_program(t_b, loop_k=1, for_hw=True):
    e_blk = t_b * 128
    s_b = e_blk // 16
    ep = NB * e_blk
    nc = bacc.Bacc("TRN2", target_bir_lowering=False, debug=False,
                   enable_asserts=False, num_devices=NCORES if for_hw else 1)

    dt_rT = nc.dram_tensor("rT", [128, ep], BF16, kind="ExternalInput")
    dt_hT = nc.dram_tensor("hT", [128, NPAD], BF16, kind="ExternalInput")
    dt_isrc = nc.dram_tensor("idx_src", [128, NB * s_b], I16, kind="ExternalInput")
    dt_dstl = nc.dram_tensor("dstl", [128, NB * t_b], F32, kind="ExternalInput")
    dt_dstlr = nc.dram_tensor("dstl_row", [1, NB * e_blk], BF16, kind="ExternalInput")
    dt_iota = nc.dram_tensor("iota", [128, 128], BF16, kind="ExternalInput")
    dt_iotac = nc.dram_tensor("iotac", [128, 1], F32, kind="ExternalInput")
    dt_ident = nc.dram_tensor("ident", [128, 128], BF16, kind="ExternalInput")
    dt_wn = nc.dram_tensor("wn", [128, 4], BF16, kind="ExternalInput")
    dt_wr = nc.dram_tensor("wr", [128, 2], BF16, kind="ExternalInput")
    dt_fcw = nc.dram_tensor("fcw", [128, 128], BF16, kind="ExternalInput")
    dt_fcb = nc.dram_tensor("fcb", [128, 128], F32, kind="ExternalInput")
    dt_consts = nc.dram_tensor("consts", [128, 8], F32, kind="ExternalInput")
    dt_out = nc.dram_tensor("out", [NB * 128, 128], F32, kind="ExternalOutput")
    dt_T3 = nc.dram_tensor("T3", [NPAD, TROW], BF16, kind="Internal")

    NG = NPAD // 512          # node-phase groups
    NPK = NPAD // 128         # node cols (slot space)
    EPK = NB * t_b            # packed edge cols

    with tile.TileContext(nc) as tc:
        with tc.tile_pool(name="const", bufs=1) as cpool:
            iota_sb = cpool.tile([128, 128], BF16)
            nc.sync.dma_start(out=iota_sb[:], in_=dt_iota.ap())
            iotac_sb = cpool.tile([128, 1], F32)
            nc.sync.dma_start(out=iotac_sb[:], in_=dt_iotac.ap())
            ident_sb = cpool.tile([128, 128], BF16)
            nc.sync.dma_start(out=ident_sb[:], in_=dt_ident.ap())
            wn_sb = cpool.tile([128, 4], BF16)
            nc.sync.dma_start(out=wn_sb[:], in_=dt_wn.ap())
            wr_sb = cpool.tile([128, 2], BF16)
            nc.sync.dma_start(out=wr_sb[:], in_=dt_wr.ap())
            fcw_sb = cpool.tile([128, 128], BF16)
            nc.sync.dma_start(out=fcw_sb[:], in_=dt_fcw.ap())
            fcb_sb = cpool.tile([128, 128], F32)
            nc.sync.dma_start(out=fcb_sb[:], in_=dt_fcb.ap())
            cst = cpool.tile([128, 8], F32)
            nc.sync.dma_start(out=cst[:], in_=dt_consts.ap())
            isrc_sb = cpool.tile([128, NB * s_b], I16)
            nc.sync.dma_start(out=isrc_sb[:], in_=dt_isrc.ap())
            dstl_sb = cpool.tile([128, NB * t_b], F32)
            nc.sync.dma_start(out=dstl_sb[:], in_=dt_dstl.ap())
            etp16 = cpool.tile([128, NPK], BF16)   # et by slot, col-major
            ones_row = cpool.tile([1, 128], BF16)
            nc.vector.memset(ones_row[:], 1.0)

            def loop_body():
                # ================== node phase: stats + T3 rows ==============
                # per 512-slot group: PE stats matmuls (lhsT=hT slice), PE
                # transposes of the same tile -> h rows -> T3 [h|1] cols;
                # LN finish batched once (single Sqrt: act-table peace), eh
                # lands in T3 col 129 via one strided column DMA; et stays
                # on-chip (etp16).
                with tc.tile_pool(name="nstat", bufs=1) as spool, \
                     tc.tile_pool(name="nwork", bufs=3) as wpool, \
                     tc.tile_pool(name="npsum", bufs=2, space="PSUM") as pp:
                    spk = spool.tile([128, NPK, 4], F32)
                    hTN = spool.tile([128, NPAD], BF16)
                    GW = 2048
                    CG = GW // 128
                    for g in range(NPAD // GW):
                        hTg = hTN[:, GW * g:GW * (g + 1)]
                        nc.sync.dma_start(out=hTg, in_=dt_hT.ap()[:, GW * g:GW * (g + 1)])
                        psS = pp.tile([128, CG, 4], F32, tag="psS")
                        for c in range(CG):
                            nc.tensor.matmul(psS[:, c, 0:3],
                                             hTg[:, 128 * c:128 * (c + 1)],
                                             wn_sb[:, 0:3], start=True, stop=True)
                        sq = wpool.tile([128, GW], BF16, tag="sq")
                        nc.scalar.activation(out=sq[:], in_=hTg, func=AF.Square)
                        for c in range(CG):
                            nc.tensor.matmul(psS[:, c, 3:4],
                                             sq[:, 128 * c:128 * (c + 1)],
                                             wn_sb[:, 0:1], start=True, stop=True)
                        nc.vector.tensor_copy(out=spk[:, CG * g:CG * (g + 1), :],
                                              in_=psS[:])
                    # batched LN finish -> eh (T3 col 129), et (on-chip bf16)
                    s1p = spk[:, :, 0]; suh = spk[:, :, 1]
                    sut = spk[:, :, 2]; s2p = spk[:, :, 3]
                    mu = spool.tile([128, NPK], F32)
                    nc.vector.tensor_scalar_mul(out=mu[:], in0=s1p, scalar1=1.0 / 128.0)
                    t0 = spool.tile([128, NPK], F32)
                    nc.vector.tensor_mul(out=t0[:], in0=mu[:], in1=mu[:])
                    nc.vector.tensor_scalar_mul(out=t0[:], in0=t0[:], scalar1=-128.0)
                    nc.vector.tensor_add(out=t0[:], in0=t0[:], in1=s2p)
                    rstd = spool.tile([128, NPK], F32)
                    nc.scalar.activation(out=rstd[:], in_=t0[:], func=AF.Sqrt, scale=1.0 / 127.0)
                    nc.vector.tensor_scalar_add(out=rstd[:], in0=rstd[:], scalar1=EPS)
                    nc.vector.reciprocal(out=rstd[:], in_=rstd[:])
                    ehc = spool.tile([128, NPK], BF16)
                    for su, sidx, cidx, dst16 in ((suh, 0, 3, ehc), (sut, 1, 4, etp16)):
                        m1 = spool.tile([128, NPK], F32, tag="m1")
                        nc.vector.tensor_scalar_mul(out=m1[:], in0=mu[:], scalar1=cst[:, sidx:sidx + 1])
                        nc.vector.tensor_sub(out=m1[:], in0=su, in1=m1[:])
                        nc.vector.tensor_mul(out=m1[:], in0=m1[:], in1=rstd[:])
                        nc.vector.tensor_scalar_add(out=m1[:], in0=m1[:], scalar1=cst[:, cidx:cidx + 1])
                        nc.scalar.activation(out=dst16[:], in_=m1[:], func=AF.Tanh)
                    for g in range(NPAD // GW):
                        psT = pp.tile([128, CG, 128], BF16, tag="psT")
                        for c in range(CG):
                            nc.tensor.transpose(psT[:, c, :],
                                                hTN[:, GW * g + 128 * c:GW * g + 128 * (c + 1)],
                                                ident_sb[:])
                        hrow = wpool.tile([128, CG, 129], BF16, tag="hrow")
                        nc.vector.tensor_copy(out=hrow[:, :, 0:128], in_=psT[:])
                        nc.vector.tensor_copy(out=hrow[:, :, 128],
                                              in_=ehc[:, CG * g:CG * (g + 1)])
                        nc.sync.dma_start(
                            out=dt_T3.ap()[GW * g:GW * (g + 1), 0:129]
                                .rearrange("(c p) w -> p c w", p=128),
                            in_=hrow[:, :, :])
                    if _PHASES == 1:
                        return

                # ================= edge phase 1: er stats ====================
                with tc.tile_pool(name="estat", bufs=1) as espool:
                    epk3 = espool.tile([128, EPK, 3], F32)
                    erp = espool.tile([128, EPK], F32)
                    with tc.tile_pool(name="ework", bufs=2) as ewpool, \
                         tc.tile_pool(name="epsum", bufs=2, space="PSUM") as epp:
                        for b in range(NB):
                            rTb = ewpool.tile([128, e_blk], BF16, tag="rTb")
                            nc.sync.dma_start(out=rTb[:], in_=dt_rT.ap()[:, b * e_blk:(b + 1) * e_blk])
                            psE = epp.tile([128, 3 * t_b], F32, tag="psE")
                            for t in range(t_b):
                                nc.tensor.matmul(psE[:, 3 * t:3 * t + 2],
                                                 rTb[:, 128 * t:128 * (t + 1)],
                                                 wr_sb[:], start=True, stop=True)
                            if b % 2 == 0:
                                nc.scalar.activation(out=rTb[:], in_=rTb[:], func=AF.Square)
                            else:
                                nc.vector.tensor_mul(out=rTb[:], in0=rTb[:], in1=rTb[:])
                            for t in range(t_b):
                                nc.tensor.matmul(psE[:, 3 * t + 2:3 * t + 3],
                                                 rTb[:, 128 * t:128 * (t + 1)],
                                                 wr_sb[:, 0:1], start=True, stop=True)
                            nc.scalar.activation(out=epk3[:, b * t_b:(b + 1) * t_b, :],
                                                 in_=psE[:], func=AF.Copy)
                    # batched er finish (strided stat views)
                    s1e = epk3[:, :, 0]; sue = epk3[:, :, 1]; s2e = epk3[:, :, 2]
                    mu = espool.tile([128, EPK], F32)
                    nc.vector.tensor_scalar_mul(out=mu[:], in0=s1e, scalar1=1.0 / 128.0)
                    t0 = espool.tile([128, EPK], F32)
                    nc.vector.tensor_mul(out=t0[:], in0=mu[:], in1=mu[:])
                    nc.vector.tensor_scalar_mul(out=t0[:], in0=t0[:], scalar1=-128.0)
                    nc.vector.tensor_add(out=t0[:], in0=t0[:], in1=s2e)
                    rstd = espool.tile([128, EPK], F32)
                    nc.scalar.activation(out=rstd[:], in_=t0[:], func=AF.Sqrt, scale=1.0 / 127.0)
                    nc.vector.tensor_scalar_add(out=rstd[:], in0=rstd[:], scalar1=EPS)
                    nc.vector.reciprocal(out=rstd[:], in_=rstd[:])
                    m1 = espool.tile([128, EPK], F32)
                    nc.vector.tensor_scalar_mul(out=m1[:], in0=mu[:], scalar1=cst[:, 2:3])
                    nc.vector.tensor_sub(out=m1[:], in0=sue, in1=m1[:])
                    nc.vector.tensor_mul(out=m1[:], in0=m1[:], in1=rstd[:])
                    nc.vector.tensor_scalar_add(out=m1[:], in0=m1[:], scalar1=cst[:, 5:6])
                    nc.scalar.activation(out=erp[:], in_=m1[:], func=AF.Tanh)
                    if _PHASES == 2:
                        nc.sync.dma_start(out=dt_out.ap()[0:128, 0:min(EPK, 128)],
                                          in_=erp[:, 0:min(EPK, 128)])
                        return

                    # ===== edge phase 2: per-block A (et expand) + B + final =====
                    with tc.tile_pool(name="gwork", bufs=3) as gpool, \
                         tc.tile_pool(name="feat", bufs=1) as fpool, \
                         tc.tile_pool(name="fin", bufs=2) as npool, \
                         tc.tile_pool(name="bpsum", bufs=2, space="PSUM") as bpp, \
                         tc.tile_pool(name="dpsum", bufs=1, space="PSUM") as dpp, \
                         tc.tile_pool(name="fpsum", bufs=2, space="PSUM") as fpp, \
                         tc.tile_pool(name="finps", bufs=1, space="PSUM") as npp:
                        featst = fpool.tile([128, NB * 129], F32)
                        obst = fpool.tile([128, NB, 128], F32)
                        nr2 = fpool.tile([128, NB], F32)
                        for b in range(NB):
                            # -- A(b): et per edge (er-independent)
                            dstlrb = gpool.tile([1, e_blk], BF16, tag="dstlrb")
                            nc.sync.dma_start(out=dstlrb[:],
                                              in_=dt_dstlr.ap()[:, b * e_blk:(b + 1) * e_blk])
                            ohT = gpool.tile([128, e_blk], BF16, tag="ohT")
                            for q in range(e_blk // 512):
                                psB = bpp.tile([128, 512], F32, tag="psB")
                                nc.tensor.matmul(psB[:], ones_row[:],
                                                 dstlrb[:, 512 * q:512 * (q + 1)],
                                                 start=True, stop=True)
                                nc.vector.tensor_scalar(out=ohT[:, 512 * q:512 * (q + 1)],
                                                        in0=psB[:],
                                                        scalar1=iotac_sb[:, 0:1],
                                                        scalar2=None, op0=OP.is_equal)
                            psD = dpp.tile([128, t_b], F32, tag="psD")
                            for t in range(t_b):
                                nc.tensor.matmul(psD[:, t:t + 1],
                                                 ohT[:, 128 * t:128 * (t + 1)],
                                                 etp16[:, b:b + 1], start=True, stop=True)
                            # -- B(b): gather + exe + scatter matmuls
                            tg = gpool.tile([128, t_b, TROW], BF16, tag="tg")
                            nc.gpsimd.dma_gather(
                                out_ap=tg[:, :, :], in_ap=dt_T3.ap(),
                                idxs_ap=isrc_sb[:, b * s_b:(b + 1) * s_b],
                                num_idxs=e_blk, num_idxs_reg=e_blk, elem_size=TROW,
                                single_packet=False)
                            # exe = exp(relu(eh_src + et_dst + er)); padded slots
                            # self-mask via out-of-range dstl (one-hot row = 0)
                            ex = gpool.tile([128, t_b], F32, tag="ex")
                            nc.vector.tensor_tensor(out=ex[:], in0=tg[:, :, 128],
                                                    in1=erp[:, b * t_b:(b + 1) * t_b],
                                                    op=OP.add)
                            nc.vector.tensor_add(out=ex[:], in0=ex[:], in1=psD[:, :])
                            nc.scalar.activation(out=ex[:], in_=ex[:], func=AF.Relu)
                            nc.scalar.activation(out=ex[:], in_=ex[:], func=AF.Exp)
                            if _PHASES in (23, 24, 25, 26) and b < 4:
                                dbg = gpool.tile([128, t_b], F32, tag="dbg")
                                if _PHASES == 23:
                                    nc.vector.tensor_copy(out=dbg[:], in_=tg[:, :, 128])
                                elif _PHASES == 24:
                                    nc.vector.tensor_copy(out=dbg[:], in_=psD[:, :])
                                elif _PHASES == 26:
                                    nc.vector.tensor_copy(out=dbg[:], in_=tg[:, 0, 0:t_b])
                                else:
                                    nc.vector.tensor_copy(out=dbg[:], in_=ex[:])
                                nc.sync.dma_start(
                                    out=dt_out.ap()[0:128, b * t_b:(b + 1) * t_b],
                                    in_=dbg[:])
                            psF = fpp.tile([128, 128], F32, tag="psF")
                            psS1 = dpp.tile([128, 1], F32, tag="psS1")
                            for t in range(t_b):
                                oh = gpool.tile([128, 128], BF16, tag="oh")
                                nc.vector.tensor_scalar(
                                    out=oh[:], in0=iota_sb[:],
                                    scalar1=dstl_sb[:, b * t_b + t:b * t_b + t + 1],
                                    scalar2=ex[:, t:t + 1],
                                    op0=OP.is_equal, op1=OP.mult)
                                nc.tensor.matmul(psF[:], oh[:], tg[:, t, 0:128],
                                                 start=(t == 0), stop=(t == t_b - 1))
                                nc.tensor.matmul(psS1[:], oh[:], wn_sb[:, 0:1],
                                                 start=(t == 0), stop=(t == t_b - 1))
                            nc.scalar.activation(out=featst[:, b * 129:b * 129 + 128],
                                                 in_=psF[:], func=AF.Copy)
                            nc.scalar.activation(out=featst[:, b * 129 + 128:(b + 1) * 129],
                                                 in_=psS1[:], func=AF.Copy)
                            # -- final(b): feat/esum, fc, norm^2
                            rs = npool.tile([128, 1], F32, tag="rs")
                            nc.vector.tensor_scalar(out=rs[:], in0=featst[:, b * 129 + 128:b * 129 + 129],
                                                    scalar1=1e-30, scalar2=None, op0=OP.max)
                            nc.vector.reciprocal(out=rs[:], in_=rs[:])
                            fs = npool.tile([128, 128], BF16, tag="fs")
                            nc.vector.tensor_scalar_mul(
                                out=fs[:], in0=featst[:, b * 129:b * 129 + 128], scalar1=rs[:])
                            psT = npp.tile([128, 128], BF16, tag="psT")
                            nc.tensor.transpose(psT[:], fs[:], ident_sb[:])
                            fT = npool.tile([128, 128], BF16, tag="fT")
                            nc.scalar.activation(out=fT[:], in_=psT[:], func=AF.Copy)
                            psO = npp.tile([128, 128], F32, tag="psO")
                            nc.tensor.matmul(psO[:], fT[:], fcw_sb[:], start=True, stop=True)
                            nc.vector.tensor_add(out=obst[:, b, :], in0=psO[:], in1=fcb_sb[:])
                            scr = npool.tile([128, 128], F32, tag="scr")
                            nc.vector.tensor_mul(out=scr[:], in0=obst[:, b, :], in1=obst[:, b, :])
                            nc.vector.reduce_sum(out=nr2[:, b:b + 1], in_=scr[:],
                                                 axis=mybir.AxisListType.X)
                        if _PHASES == 3:
                            nc.sync.dma_start(out=dt_out.ap()[0:128, 0:128],
                                              in_=featst[:, 0:128])
                            return
                        # ============== batched L2 normalize + out ==============
                        nrm = fpool.tile([128, NB], F32)
                        nc.scalar.activation(out=nrm[:], in_=nr2[:], func=AF.Sqrt)
                        nc.vector.tensor_scalar(out=nrm[:], in0=nrm[:],
                                                scalar1=1e-12, scalar2=None, op0=OP.max)
                        nc.vector.reciprocal(out=nrm[:], in_=nrm[:])
                        for b in range(NB):
                            ob = npool.tile([128, 128], F32, tag="ob")
                            nc.vector.tensor_scalar_mul(out=ob[:], in0=obst[:, b, :],
                                                        scalar1=nrm[:, b:b + 1])
                            nc.sync.dma_start(out=dt_out.ap()[b * 128:(b + 1) * 128, :], in_=ob[:])

            if loop_k == 1:
                loop_body()
            else:
                with tc.For_i(0, loop_k, 1):
                    loop_body()

    nc.compile()
    if for_hw:
        nc.m = get_hw_module(nc.m)
    return nc


# ------------------------------------------------------------------- runner
class Runner:
    def __init__(self, nc, n_cores=NCORES):
        import jax
        from concourse.bass2jax import (_bass_exec_p, partition_id_tensor,
                                        install_neuronx_cc_hook)
        from jax.sharding import Mesh, PartitionSpec, NamedSharding
        from jax.experimental.shard_map import shard_map
        install_neuronx_cc_hook()
        self.jax = jax
        self.n_cores = n_cores
        pname = nc.partition_id_tensor.name if nc.partition_id_tensor else None
        in_names, out_names, out_avals = [], [], []
        for alloc in nc.m.functions[0].allocations:
            if not isinstance(alloc, mybir.MemoryLocationSet):
                continue
            name = alloc.memorylocations[0].name
            if alloc.kind == "ExternalInput":
                if name != pname:
                    in_names.append(name)
            elif alloc.kind == "ExternalOutput":
                out_names.append(name)
                out_avals.append(jax.core.ShapedArray(
                    tuple(alloc.tensor_shape), mybir.dt.np(alloc.dtype)))
        self.in_names, self.out_names, self.out_avals = in_names, out_names, out_avals
        n_params = len(in_names)
        all_in = list(in_names) + list(out_names)
        if pname is not None:
            all_in.append(pname)

        def _body(*args):
            operands = list(args)
            if pname is not None:
                operands.append(partition_id_tensor())
            return tuple(_bass_exec_p.bind(
                *operands, out_avals=tuple(out_avals), in_names=tuple(all_in),
                out_names=tuple(out_names), lowering_input_output_aliases=(),
                sim_require_finite=True, sim_require_nnan=True, nc=nc))

        devices = jax.devices()[:n_cores]
        self.mesh = Mesh(np.asarray(devices), ("core",))
        self.sharding = NamedSharding(self.mesh, PartitionSpec("core"))
        donate = tuple(range(n_params, n_params + len(out_names)))
        self.fn = jax.jit(shard_map(
            _body, mesh=self.mesh,
            in_specs=(PartitionSpec("core"),) * (n_params + len(out_names)),
            out_specs=(PartitionSpec("core"),) * len(out_names),
            check_rep=False), donate_argnums=donate, keep_unused=True)

    def put_inputs(self, in_maps):
        return [self.jax.device_put(
            np.concatenate([np.asarray(in_maps[c][nm]) for c in range(self.n_cores)], axis=0),
            self.sharding) for nm in self.in_names]

    def put_zeros(self):
        return [self.jax.device_put(
            np.zeros((self.n_cores * a.shape[0], *a.shape[1:]), a.dtype), self.sharding)
            for a in self.out_avals]

    def run(self, dev_in, dev_zeros):
        outs = self.fn(*dev_in, *dev_zeros)
        self.jax.block_until_ready(outs)
        return outs

    def unpack(self, outs):
        return [{nm: np.asarray(outs[i]).reshape(self.n_cores, *self.out_avals[i].shape)[c]
                 for i, nm in enumerate(self.out_names)} for c in range(self.n_cores)]


_CACHE = {}


def _get_runner(t_b, loop_k=1):
    key = (t_b, loop_k)
    if key not in _CACHE:
        nc = build_program(t_b, loop_k)
        _CACHE[key] = Runner(nc)
    return _CACHE[key]


def kernel(**inputs):
    in_maps, meta = _host_prep(**inputs)
    r = _get_runner(meta["t_b"], 1)
    dev = r.put_inputs(in_maps)
    res = r.unpack(r.run(dev, r.put_zeros()))
    bin_of, loc_of = meta["bin_of"], meta["loc_of"]
    slots = bin_of.astype(np.int64) * 128 + loc_of      # global slot per node
    allout = np.concatenate([res[k]["out"] for k in range(NCORES)], axis=0)
    return np.ascontiguousarray(allout[slots])


# revision 31
# speedup vs baseline: 3.1640x; 1.5622x over previous
"""GAT layer kernel for Trainium2, 8 NeuronCores, edge/node-parallel.

Strategy v2 (degree-balanced node bins, bf16 streams, no dst gather):
  - Host: greedy LPT-pack nodes into 160 bins of <=128 nodes with near-equal
    total in-degree -> every (core, block) has ~E/160 edges, t_b uniform,
    padding ~2.5%.  Edges grouped by dst bin; dstl = local index in bin.
    All node data lives in bin-slot space, rotated per core so each core's
    own 20 bins occupy slots 0..2559 (SPMD program, per-core data).
  - Node phase: stream (permuted) h^T in bf16; per 512-node group compute LN
    stats via PE matmuls, PE-transpose the same tile back to h rows, and emit
    T3 rows [h(128) | 1 | eh | pad] in bf16 (512 B rows; descriptor cost is
    flat below 512B so the row rides free).  et stays on-chip as a bf16
    [128, 160] tile whose column b holds the et of block b's 128 dst nodes.
  - Edge phase per block: er = tanh(LN(r)@w) from a bf16 r^T stream; ONE
    dma_gather of T3 rows by src slot (512 B); et per edge via
    partition_broadcast of the slot-ordered dstl row + is_equal -> transposed
    one-hot -> 1-col matmuls against the resident et column (no dst gather);
    softmax without max-subtraction (exp(relu(x)) == max(1,exp(x)));
    bf16 scaled one-hot in ONE tensor_scalar; PSUM-accumulated bf16 matmul
    onehot^T @ [h|1] gives feat and esum together.
  - Final: feat/esum, feat @ fc_w + b, row L2 normalize, DMA out.
"""

import os
import sys

sys.path.insert(0, "/opt/trn_rl_repo")

_PHASES = int(os.environ.get("KPHASES", "4"))

import heapq

import ml_dtypes
import numpy as np

import concourse.bacc as bacc
import concourse.bass as bass
import concourse.mybir as mybir
import concourse.tile as tile
from concourse.bass_interp import get_hw_module

F32 = mybir.dt.float32
BF16 = mybir.dt.bfloat16
I16 = mybir.dt.int16
AF = mybir.ActivationFunctionType
OP = mybir.AluOpType
NPBF = ml_dtypes.bfloat16

N = 20000
E = 640000
D = 128
NCORES = 8
EPS = 1e-6
NPAD = 20480          # slots: 160 bins * 128
NBIN = 160            # global 128-node bins
NB = NBIN // NCORES   # 20 blocks per core
NSLC = NPAD // NCORES  # slots per core (rotation unit)
TROW = 256            # T3 row: [h(128) | 1 | eh | pad] bf16 (512B)


# ----------------------------------------------------------------- host prep
def _host_prep(h, r, src, dst, hn_a, hn_b, tn_a, tn_b, rn_a, rn_b,
               head_w, tail_w, rel_w, fc_w, fc_b):
    h = np.asarray(h, np.float32); r = np.asarray(r, np.float32)
    src = np.asarray(src, np.int32); dst = np.asarray(dst, np.int32)

    u_h = np.asarray(hn_a, np.float32) * np.asarray(head_w, np.float32)
    u_t = np.asarray(tn_a, np.float32) * np.asarray(tail_w, np.float32)
    u_r = np.asarray(rn_a, np.float32) * np.asarray(rel_w, np.float32)
    s_uh = float(u_h.sum()); s_ut = float(u_t.sum()); s_ur = float(u_r.sum())
    c_h = float((np.asarray(hn_b, np.float32) * head_w).sum())
    c_t = float((np.asarray(tn_b, np.float32) * tail_w).sum())
    c_r = float((np.asarray(rn_b, np.float32) * rel_w).sum())

    # ---- LPT bin packing: 160 bins, <=128 nodes, balanced in-degree
    deg = np.bincount(dst, minlength=N).astype(np.int64)
    order = np.argsort(-deg, kind="stable")
    bin_of = np.empty(N, np.int32)
    loc_of = np.empty(N, np.int32)
    bin_cnt = np.zeros(NBIN, np.int32)
    bin_edges = np.zeros(NBIN, np.int64)
    heap = [(0, b) for b in range(NBIN)]
    heapq.heapify(heap)
    for n in order:
        while True:
            e_b, b = heapq.heappop(heap)
            if bin_cnt[b] < 128:
                break
        bin_of[n] = b
        loc_of[n] = bin_cnt[b]
        bin_cnt[b] += 1
        bin_edges[b] = e_b + deg[n]
        if bin_cnt[b] < 128:
            heapq.heappush(heap, (int(bin_edges[b]), b))
    t_b = max(1, int(-(-int(bin_edges.max()) // 128)))
    e_blk = t_b * 128
    s_b = e_blk // 16
    ep = NB * e_blk

    slot_of = bin_of.astype(np.int64) * 128 + loc_of   # node -> global slot
    node_of = np.zeros(NPAD, np.int64)                 # slot -> node (pad: 0)
    node_of[slot_of] = np.arange(N)

    # ---- edges grouped by dst bin
    eb = bin_of[dst]
    perm = np.argsort(eb, kind="stable")
    src_s = src[perm]; dst_s = dst[perm]
    counts = np.bincount(eb, minlength=NBIN)
    cum = np.concatenate([[0], np.cumsum(counts)])

    # ---- replicated tensors
    h_slot = h[node_of]                                # [NPAD, D] slot space
    hT_slot = np.ascontiguousarray(h_slot.T.astype(NPBF))  # [128, NPAD]
    iota = np.broadcast_to(np.arange(128, dtype=np.float32),
                           (128, 128)).astype(NPBF).copy()
    iotac = np.arange(128, dtype=np.float32).reshape(128, 1)
    ident = np.eye(128, dtype=np.float32).astype(NPBF)
    wn = np.zeros((128, 4), np.float32)
    wn[:, 0] = 1.0; wn[:, 1] = u_h; wn[:, 2] = u_t
    wn = wn.astype(NPBF)
    wr = np.zeros((128, 2), np.float32)
    wr[:, 0] = 1.0; wr[:, 1] = u_r
    wr = wr.astype(NPBF)
    fcw = np.ascontiguousarray(np.asarray(fc_w, np.float32)).astype(NPBF)
    fcb = np.broadcast_to(np.asarray(fc_b, np.float32), (128, 128)).copy()
    consts = np.zeros((128, 8), np.float32)
    consts[:, 0] = s_uh; consts[:, 1] = s_ut; consts[:, 2] = s_ur
    consts[:, 3] = c_h; consts[:, 4] = c_t; consts[:, 5] = c_r

    rep = {"iota": iota, "iotac": iotac, "ident": ident, "wn": wn, "wr": wr,
           "fcw": fcw, "fcb": fcb, "consts": consts}

    in_maps = []
    for k in range(NCORES):
        src16 = np.zeros((NB, e_blk), np.int16)
        dstl = np.full((NB, e_blk), 200.0, np.float32)
        rcol = np.zeros((NB, e_blk), np.int64)
        for j in range(NB):
            b = k * NB + j
            e0, e1 = int(cum[b]), int(cum[b + 1])
            cnt = e1 - e0
            # src slot in core-k-rotated space
            src16[j, :cnt] = (slot_of[src_s[e0:e1]] - NSLC * k) % NPAD
            dstl[j, :cnt] = loc_of[dst_s[e0:e1]]
            rcol[j, :cnt] = perm[e0:e1]
        rT = np.ascontiguousarray(r[rcol.reshape(-1)].T).astype(NPBF)
        hT = np.ascontiguousarray(np.roll(hT_slot, -NSLC * k, axis=1))

        def wrap16(a):
            blk = a.reshape(NB, s_b, 16).transpose(0, 2, 1)    # [NB,16,s_b]
            out = np.tile(blk, (1, 8, 1))                      # [NB,128,s_b]
            return np.ascontiguousarray(
                out.transpose(1, 0, 2).reshape(128, NB * s_b))

        def pk(a):
            x = a.reshape(NB, t_b, 128).transpose(2, 0, 1)     # [128, NB, t_b]
            return np.ascontiguousarray(x.reshape(128, NB * t_b))

        in_maps.append(dict(rep, rT=rT, hT=hT, idx_src=wrap16(src16),
                            dstl=pk(dstl),
                            dstl_row=dstl.reshape(1, NB * e_blk).astype(NPBF)))
    meta = dict(t_b=t_b, e_blk=e_blk, s_b=s_b, ep=ep,
                bin_of=bin_of, loc_of=loc_of)
    return in_maps, meta


# ------------------------------------------------------------ device program
def build_program(t_b, loop_k=1, for_hw=True):
    e_blk = t_b * 128
    s_b = e_blk // 16
    ep = NB * e_blk
    nc = bacc.Bacc("TRN2", target_bir_lowering=False, debug=False,
                   enable_asserts=False, num_devices=NCORES if for_hw else 1)

    dt_rT = nc.dram_tensor("rT", [128, ep], BF16, kind="ExternalInput")
    dt_hT = nc.dram_tensor("hT", [128, NPAD], BF16, kind="ExternalInput")
    dt_isrc = nc.dram_tensor("idx_src", [128, NB * s_b], I16, kind="ExternalInput")
    dt_dstl = nc.dram_tensor("dstl", [128, NB * t_b], F32, kind="ExternalInput")
    dt_dstlr = nc.dram_tensor("dstl_row", [1, NB * e_blk], BF16, kind="ExternalInput")
    dt_iota = nc.dram_tensor("iota", [128, 128], BF16, kind="ExternalInput")
    dt_iotac = nc.dram_tensor("iotac", [128, 1], F32, kind="ExternalInput")
    dt_ident = nc.dram_tensor("ident", [128, 128], BF16, kind="ExternalInput")
    dt_wn = nc.dram_tensor("wn", [128, 4], BF16, kind="ExternalInput")
    dt_wr = nc.dram_tensor("wr", [128, 2], BF16, kind="ExternalInput")
    dt_fcw = nc.dram_tensor("fcw", [128, 128], BF16, kind="ExternalInput")
    dt_fcb = nc.dram_tensor("fcb", [128, 128], F32, kind="ExternalInput")
    dt_consts = nc.dram_tensor("consts", [128, 8], F32, kind="ExternalInput")
    dt_out = nc.dram_tensor("out", [NB * 128, 128], F32, kind="ExternalOutput")
    dt_T3 = nc.dram_tensor("T3", [NPAD, TROW], BF16, kind="Internal")

    NG = NPAD // 512          # node-phase groups
    NPK = NPAD // 128         # node cols (slot space)
    EPK = NB * t_b            # packed edge cols

    with tile.TileContext(nc) as tc:
        with tc.tile_pool(name="const", bufs=1) as cpool:
            iota_sb = cpool.tile([128, 128], BF16)
            nc.sync.dma_start(out=iota_sb[:], in_=dt_iota.ap())
            iotac_sb = cpool.tile([128, 1], F32)
            nc.sync.dma_start(out=iotac_sb[:], in_=dt_iotac.ap())
            ident_sb = cpool.tile([128, 128], BF16)
            nc.sync.dma_start(out=ident_sb[:], in_=dt_ident.ap())
            wn_sb = cpool.tile([128, 4], BF16)
            nc.sync.dma_start(out=wn_sb[:], in_=dt_wn.ap())
            wr_sb = cpool.tile([128, 2], BF16)
            nc.sync.dma_start(out=wr_sb[:], in_=dt_wr.ap())
            fcw_sb = cpool.tile([128, 128], BF16)
            nc.sync.dma_start(out=fcw_sb[:], in_=dt_fcw.ap())
            fcb_sb = cpool.tile([128, 128], F32)
            nc.sync.dma_start(out=fcb_sb[:], in_=dt_fcb.ap())
            cst = cpool.tile([128, 8], F32)
            nc.sync.dma_start(out=cst[:], in_=dt_consts.ap())
            isrc_sb = cpool.tile([128, NB * s_b], I16)
            nc.sync.dma_start(out=isrc_sb[:], in_=dt_isrc.ap())
            dstl_sb = cpool.tile([128, NB * t_b], F32)
            nc.sync.dma_start(out=dstl_sb[:], in_=dt_dstl.ap())
            etp16 = cpool.tile([128, NPK], BF16)   # et by slot, col-major
            ones_row = cpool.tile([1, 128], BF16)
            nc.vector.memset(ones_row[:], 1.0)

            def loop_body():
                # ================== node phase: stats + T3 rows ==============
                # per 512-slot group: PE stats matmuls (lhsT=hT slice), PE
                # transposes of the same tile -> h rows -> T3 [h|1] cols;
                # LN finish batched once (single Sqrt: act-table peace), eh
                # lands in T3 col 129 via one strided column DMA; et stays
                # on-chip (etp16).
                with tc.tile_pool(name="nstat", bufs=1) as spool, \
                     tc.tile_pool(name="nwork", bufs=3) as wpool, \
                     tc.tile_pool(name="npsum", bufs=2, space="PSUM") as pp:
                    spk = spool.tile([128, NPK, 4], F32)
                    hTN = spool.tile([128, NPAD], BF16)
                    GW = 2048
                    CG = GW // 128
                    for g in range(NPAD // GW):
                        hTg = hTN[:, GW * g:GW * (g + 1)]
                        nc.sync.dma_start(out=hTg, in_=dt_hT.ap()[:, GW * g:GW * (g + 1)])
                        psS = pp.tile([128, CG, 4], F32, tag="psS")
                        for c in range(CG):
                            nc.tensor.matmul(psS[:, c, 0:3],
                                             hTg[:, 128 * c:128 * (c + 1)],
                                             wn_sb[:, 0:3], start=True, stop=True)
                        sq = wpool.tile([128, GW], BF16, tag="sq")
                        nc.scalar.activation(out=sq[:], in_=hTg, func=AF.Square)
                        for c in range(CG):
                            nc.tensor.matmul(psS[:, c, 3:4],
                                             sq[:, 128 * c:128 * (c + 1)],
                                             wn_sb[:, 0:1], start=True, stop=True)
                        nc.vector.tensor_copy(out=spk[:, CG * g:CG * (g + 1), :],
                                              in_=psS[:])
                    # batched LN finish -> eh (T3 col 129), et (on-chip bf16)
                    s1p = spk[:, :, 0]; suh = spk[:, :, 1]
                    sut = spk[:, :, 2]; s2p = spk[:, :, 3]
                    mu = spool.tile([128, NPK], F32)
                    nc.vector.tensor_scalar_mul(out=mu[:], in0=s1p, scalar1=1.0 / 128.0)
                    t0 = spool.tile([128, NPK], F32)
                    nc.vector.tensor_mul(out=t0[:], in0=mu[:], in1=mu[:])
                    nc.vector.tensor_scalar_mul(out=t0[:], in0=t0[:], scalar1=-128.0)
                    nc.vector.tensor_add(out=t0[:], in0=t0[:], in1=s2p)
                    rstd = spool.tile([128, NPK], F32)
                    nc.scalar.activation(out=rstd[:], in_=t0[:], func=AF.Sqrt, scale=1.0 / 127.0)
                    nc.vector.tensor_scalar_add(out=rstd[:], in0=rstd[:], scalar1=EPS)
                    nc.vector.reciprocal(out=rstd[:], in_=rstd[:])
                    ehc = spool.tile([128, NPK], BF16)
                    for su, sidx, cidx, dst16 in ((suh, 0, 3, ehc), (sut, 1, 4, etp16)):
                        m1 = spool.tile([128, NPK], F32, tag="m1")
                        nc.vector.tensor_scalar_mul(out=m1[:], in0=mu[:], scalar1=cst[:, sidx:sidx + 1])
                        nc.vector.tensor_sub(out=m1[:], in0=su, in1=m1[:])
                        nc.vector.tensor_mul(out=m1[:], in0=m1[:], in1=rstd[:])
                        nc.vector.tensor_scalar_add(out=m1[:], in0=m1[:], scalar1=cst[:, cidx:cidx + 1])
                        nc.scalar.activation(out=dst16[:], in_=m1[:], func=AF.Tanh)
                    for g in range(NPAD // GW):
                        psT = pp.tile([128, CG, 128], BF16, tag="psT")
                        for c in range(CG):
                            nc.tensor.transpose(psT[:, c, :],
                                                hTN[:, GW * g + 128 * c:GW * g + 128 * (c + 1)],
                                                ident_sb[:])
                        hrow = wpool.tile([128, CG, 130], BF16, tag="hrow")
                        nc.vector.tensor_copy(out=hrow[:, :, 0:128], in_=psT[:])
                        nc.vector.memset(hrow[:, :, 128:129], 1.0)
                        nc.vector.tensor_copy(out=hrow[:, :, 129],
                                              in_=ehc[:, CG * g:CG * (g + 1)])
                        nc.sync.dma_start(
                            out=dt_T3.ap()[GW * g:GW * (g + 1), 0:130]
                                .rearrange("(c p) w -> p c w", p=128),
                            in_=hrow[:, :, :])
                    if _PHASES == 1:
                        return

                # ================= edge phase 1: er stats ====================
                with tc.tile_pool(name="estat", bufs=1) as espool:
                    epk3 = espool.tile([128, EPK, 3], F32)
                    erp = espool.tile([128, EPK], F32)
                    with tc.tile_pool(name="ework", bufs=2) as ewpool, \
                         tc.tile_pool(name="epsum", bufs=2, space="PSUM") as epp:
                        for b in range(NB):
                            rTb = ewpool.tile([128, e_blk], BF16, tag="rTb")
                            nc.sync.dma_start(out=rTb[:], in_=dt_rT.ap()[:, b * e_blk:(b + 1) * e_blk])
                            psE = epp.tile([128, 3 * t_b], F32, tag="psE")
                            for t in range(t_b):
                                nc.tensor.matmul(psE[:, 3 * t:3 * t + 2],
                                                 rTb[:, 128 * t:128 * (t + 1)],
                                                 wr_sb[:], start=True, stop=True)
                            if b % 2 == 0:
                                nc.scalar.activation(out=rTb[:], in_=rTb[:], func=AF.Square)
                            else:
                                nc.vector.tensor_mul(out=rTb[:], in0=rTb[:], in1=rTb[:])
                            for t in range(t_b):
                                nc.tensor.matmul(psE[:, 3 * t + 2:3 * t + 3],
                                                 rTb[:, 128 * t:128 * (t + 1)],
                                                 wr_sb[:, 0:1], start=True, stop=True)
                            nc.scalar.activation(out=epk3[:, b * t_b:(b + 1) * t_b, :],
                                                 in_=psE[:], func=AF.Copy)
                    # batched er finish (strided stat views)
                    s1e = epk3[:, :, 0]; sue = epk3[:, :, 1]; s2e = epk3[:, :, 2]
                    mu = espool.tile([128, EPK], F32)
                    nc.vector.tensor_scalar_mul(out=mu[:], in0=s1e, scalar1=1.0 / 128.0)
                    t0 = espool.tile([128, EPK], F32)
                    nc.vector.tensor_mul(out=t0[:], in0=mu[:], in1=mu[:])
                    nc.vector.tensor_scalar_mul(out=t0[:], in0=t0[:], scalar1=-128.0)
                    nc.vector.tensor_add(out=t0[:], in0=t0[:], in1=s2e)
                    rstd = espool.tile([128, EPK], F32)
                    nc.scalar.activation(out=rstd[:], in_=t0[:], func=AF.Sqrt, scale=1.0 / 127.0)
                    nc.vector.tensor_scalar_add(out=rstd[:], in0=rstd[:], scalar1=EPS)
                    nc.vector.reciprocal(out=rstd[:], in_=rstd[:])
                    m1 = espool.tile([128, EPK], F32)
                    nc.vector.tensor_scalar_mul(out=m1[:], in0=mu[:], scalar1=cst[:, 2:3])
                    nc.vector.tensor_sub(out=m1[:], in0=sue, in1=m1[:])
                    nc.vector.tensor_mul(out=m1[:], in0=m1[:], in1=rstd[:])
                    nc.vector.tensor_scalar_add(out=m1[:], in0=m1[:], scalar1=cst[:, 5:6])
                    nc.scalar.activation(out=erp[:], in_=m1[:], func=AF.Tanh)
                    if _PHASES == 2:
                        nc.sync.dma_start(out=dt_out.ap()[0:128, 0:min(EPK, 128)],
                                          in_=erp[:, 0:min(EPK, 128)])
                        return

                    # ===== edge phase 2: per-block A (et expand) + B + final =====
                    with tc.tile_pool(name="gwork", bufs=3) as gpool, \
                         tc.tile_pool(name="feat", bufs=1) as fpool, \
                         tc.tile_pool(name="fin", bufs=2) as npool, \
                         tc.tile_pool(name="bpsum", bufs=2, space="PSUM") as bpp, \
                         tc.tile_pool(name="dpsum", bufs=1, space="PSUM") as dpp, \
                         tc.tile_pool(name="fpsum", bufs=2, space="PSUM") as fpp, \
                         tc.tile_pool(name="finps", bufs=1, space="PSUM") as npp:
                        featst = fpool.tile([128, NB * 129], F32)
                        obst = fpool.tile([128, NB, 128], F32)
                        nr2 = fpool.tile([128, NB], F32)
                        for b in range(NB):
                            # -- A(b): et per edge (er-independent)
                            dstlrb = gpool.tile([1, e_blk], BF16, tag="dstlrb")
                            nc.sync.dma_start(out=dstlrb[:],
                                              in_=dt_dstlr.ap()[:, b * e_blk:(b + 1) * e_blk])
                            ohT = gpool.tile([128, e_blk], BF16, tag="ohT")
                            for q in range(e_blk // 512):
                                psB = bpp.tile([128, 512], F32, tag="psB")
                                nc.tensor.matmul(psB[:], ones_row[:],
                                                 dstlrb[:, 512 * q:512 * (q + 1)],
                                                 start=True, stop=True)
                                nc.vector.tensor_scalar(out=ohT[:, 512 * q:512 * (q + 1)],
                                                        in0=psB[:],
                                                        scalar1=iotac_sb[:, 0:1],
                                                        scalar2=None, op0=OP.is_equal)
                            psD = dpp.tile([128, t_b], F32, tag="psD")
                            for t in range(t_b):
                                nc.tensor.matmul(psD[:, t:t + 1],
                                                 ohT[:, 128 * t:128 * (t + 1)],
                                                 etp16[:, b:b + 1], start=True, stop=True)
                            # -- B(b): gather + exe + scatter matmuls
                            tg = gpool.tile([128, t_b, TROW], BF16, tag="tg")
                            nc.gpsimd.dma_gather(
                                out_ap=tg[:, :, :], in_ap=dt_T3.ap(),
                                idxs_ap=isrc_sb[:, b * s_b:(b + 1) * s_b],
                                num_idxs=e_blk, num_idxs_reg=e_blk, elem_size=TROW,
                                single_packet=False)
                            # exe = exp(relu(eh_src + et_dst + er)); padded slots
                            # self-mask via out-of-range dstl (one-hot row = 0)
                            ex = gpool.tile([128, t_b], F32, tag="ex")
                            nc.vector.tensor_tensor(out=ex[:], in0=tg[:, :, 129],
                                                    in1=erp[:, b * t_b:(b + 1) * t_b],
                                                    op=OP.add)
                            nc.vector.tensor_add(out=ex[:], in0=ex[:], in1=psD[:, :])
                            nc.scalar.activation(out=ex[:], in_=ex[:], func=AF.Relu)
                            nc.scalar.activation(out=ex[:], in_=ex[:], func=AF.Exp)
                            if _PHASES in (23, 24, 25, 26) and b < 4:
                                dbg = gpool.tile([128, t_b], F32, tag="dbg")
                                if _PHASES == 23:
                                    nc.vector.tensor_copy(out=dbg[:], in_=tg[:, :, 128])
                                elif _PHASES == 24:
                                    nc.vector.tensor_copy(out=dbg[:], in_=psD[:, :])
                                elif _PHASES == 26:
                                    nc.vector.tensor_copy(out=dbg[:], in_=tg[:, 0, 0:t_b])
                                else:
                                    nc.vector.tensor_copy(out=dbg[:], in_=ex[:])
                                nc.sync.dma_start(
                                    out=dt_out.ap()[0:128, b * t_b:(b + 1) * t_b],
                                    in_=dbg[:])
                            psF = fpp.tile([128, 129], F32, tag="psF")
                            for t in range(t_b):
                                oh = gpool.tile([128, 128], BF16, tag="oh")
                                nc.vector.tensor_scalar(
                                    out=oh[:], in0=iota_sb[:],
                                    scalar1=dstl_sb[:, b * t_b + t:b * t_b + t + 1],
                                    scalar2=ex[:, t:t + 1],
                                    op0=OP.is_equal, op1=OP.mult)
                                nc.tensor.matmul(psF[:], oh[:], tg[:, t, 0:129],
                                                 start=(t == 0), stop=(t == t_b - 1))
                            nc.scalar.activation(out=featst[:, b * 129:(b + 1) * 129],
                                                 in_=psF[:], func=AF.Copy)
                            # -- final(b): feat/esum, fc, norm^2
                            rs = npool.tile([128, 1], F32, tag="rs")
                            nc.vector.tensor_scalar(out=rs[:], in0=featst[:, b * 129 + 128:b * 129 + 129],
                                                    scalar1=1e-30, scalar2=None, op0=OP.max)
                            nc.vector.reciprocal(out=rs[:], in_=rs[:])
                            fs = npool.tile([128, 128], BF16, tag="fs")
                            nc.vector.tensor_scalar_mul(
                                out=fs[:], in0=featst[:, b * 129:b * 129 + 128], scalar1=rs[:])
                            psT = npp.tile([128, 128], BF16, tag="psT")
                            nc.tensor.transpose(psT[:], fs[:], ident_sb[:])
                            fT = npool.tile([128, 128], BF16, tag="fT")
                            nc.scalar.activation(out=fT[:], in_=psT[:], func=AF.Copy)
                            psO = npp.tile([128, 128], F32, tag="psO")
                            nc.tensor.matmul(psO[:], fT[:], fcw_sb[:], start=True, stop=True)
                            nc.vector.tensor_add(out=obst[:, b, :], in0=psO[:], in1=fcb_sb[:])
                            scr = npool.tile([128, 128], F32, tag="scr")
                            nc.vector.tensor_mul(out=scr[:], in0=obst[:, b, :], in1=obst[:, b, :])
                            nc.vector.reduce_sum(out=nr2[:, b:b + 1], in_=scr[:],
                                                 axis=mybir.AxisListType.X)
                        if _PHASES == 3:
                            nc.sync.dma_start(out=dt_out.ap()[0:128, 0:128],
                                              in_=featst[:, 0:128])
                            return
                        # ============== batched L2 normalize + out ==============
                        nrm = fpool.tile([128, NB], F32)
                        nc.scalar.activation(out=nrm[:], in_=nr2[:], func=AF.Sqrt)
                        nc.vector.tensor_scalar(out=nrm[:], in0=nrm[:],
                                                scalar1=1e-12, scalar2=None, op0=OP.max)
                        nc.vector.reciprocal(out=nrm[:], in_=nrm[:])
                        for b in range(NB):
                            ob = npool.tile([128, 128], F32, tag="ob")
                            nc.vector.tensor_scalar_mul(out=ob[:], in0=obst[:, b, :],
                                                        scalar1=nrm[:, b:b + 1])
                            nc.sync.dma_start(out=dt_out.ap()[b * 128:(b + 1) * 128, :], in_=ob[:])

            if loop_k == 1:
                loop_body()
            else:
                with tc.For_i(0, loop_k, 1):
                    loop_body()

    nc.compile()
    if for_hw:
        nc.m = get_hw_module(nc.m)
    return nc


# ------------------------------------------------------------------- runner
class Runner:
    def __init__(self, nc, n_cores=NCORES):
        import jax
        from concourse.bass2jax import (_bass_exec_p, partition_id_tensor,
                                        install_neuronx_cc_hook)
        from jax.sharding import Mesh, PartitionSpec, NamedSharding
        from jax.experimental.shard_map import shard_map
        install_neuronx_cc_hook()
        self.jax = jax
        self.n_cores = n_cores
        pname = nc.partition_id_tensor.name if nc.partition_id_tensor else None
        in_names, out_names, out_avals = [], [], []
        for alloc in nc.m.functions[0].allocations:
            if not isinstance(alloc, mybir.MemoryLocationSet):
                continue
            name = alloc.memorylocations[0].name
            if alloc.kind == "ExternalInput":
                if name != pname:
                    in_names.append(name)
            elif alloc.kind == "ExternalOutput":
                out_names.append(name)
                out_avals.append(jax.core.ShapedArray(
                    tuple(alloc.tensor_shape), mybir.dt.np(alloc.dtype)))
        self.in_names, self.out_names, self.out_avals = in_names, out_names, out_avals
        n_params = len(in_names)
        all_in = list(in_names) + list(out_names)
        if pname is not None:
            all_in.append(pname)

        def _body(*args):
            operands = list(args)
            if pname is not None:
                operands.append(partition_id_tensor())
            return tuple(_bass_exec_p.bind(
                *operands, out_avals=tuple(out_avals), in_names=tuple(all_in),
                out_names=tuple(out_names), lowering_input_output_aliases=(),
                sim_require_finite=True, sim_require_nnan=True, nc=nc))

        devices = jax.devices()[:n_cores]
        self.mesh = Mesh(np.asarray(devices), ("core",))
        self.sharding = NamedSharding(self.mesh, PartitionSpec("core"))
        donate = tuple(range(n_params, n_params + len(out_names)))
        self.fn = jax.jit(shard_map(
            _body, mesh=self.mesh,
            in_specs=(PartitionSpec("core"),) * (n_params + len(out_names)),
            out_specs=(PartitionSpec("core"),) * len(out_names),
            check_rep=False), donate_argnums=donate, keep_unused=True)

    def put_inputs(self, in_maps):
        return [self.jax.device_put(
            np.concatenate([np.asarray(in_maps[c][nm]) for c in range(self.n_cores)], axis=0),
            self.sharding) for nm in self.in_names]

    def put_zeros(self):
        return [self.jax.device_put(
            np.zeros((self.n_cores * a.shape[0], *a.shape[1:]), a.dtype), self.sharding)
            for a in self.out_avals]

    def run(self, dev_in, dev_zeros):
        outs = self.fn(*dev_in, *dev_zeros)
        self.jax.block_until_ready(outs)
        return outs

    def unpack(self, outs):
        return [{nm: np.asarray(outs[i]).reshape(self.n_cores, *self.out_avals[i].shape)[c]
                 for i, nm in enumerate(self.out_names)} for c in range(self.n_cores)]


_CACHE = {}


def _get_runner(t_b, loop_k=1):
    key = (t_b, loop_k)
    if key not in _CACHE:
        nc = build_program(t_b, loop_k)
        _CACHE[key] = Runner(nc)
    return _CACHE[key]


def kernel(**inputs):
    in_maps, meta = _host_prep(**inputs)
    r = _get_runner(meta["t_b"], 1)
    dev = r.put_inputs(in_maps)
    res = r.unpack(r.run(dev, r.put_zeros()))
    bin_of, loc_of = meta["bin_of"], meta["loc_of"]
    slots = bin_of.astype(np.int64) * 128 + loc_of      # global slot per node
    allout = np.concatenate([res[k]["out"] for k in range(NCORES)], axis=0)
    return np.ascontiguousarray(allout[slots])
